# revision 7
# baseline (speedup 1.0000x reference)
"""Trainium2 Bass kernel for gnn_message_passing (nn_BFR_28089086116615).

Sharding: receiver axis i (G=4096 -> 8 cores x 512). Host pre-transposes the
edge matrices and folds the {coef, 1} gate weights in bf16: wT[j, i]. On
device, sigma^T is computed natively in [j-partition, i-free] layout (ACT
sigmoid, per-partition bias = s_src[j-chunk], input = broadcast s_dst row),
gated by wT on DVE (bf16 2x), and contracted on PE with stationary weights
[h | 1] so the receiver rowsum rides along as an extra psum row. BatchNorm is
per-gene -> fully local; one AllGather (both layouts) of normalized h between
the two message-passing blocks.
"""
import sys
sys.path.insert(0, "/opt/trn_rl_repo")
import numpy as np
import ml_dtypes

import concourse.bass as bass
import concourse.bacc as bacc
import concourse.mybir as mybir
import concourse.tile as tile
from concourse.bass_utils import run_bass_kernel_spmd

NC = 8
B, G, NI, H, NO = 2, 4096, 8, 32, 32
GL = G // NC              # 512 local receivers per core
LCH = GL // 128           # 4 local chunks
NCH = G // 128            # 32 global j-chunks
QC = 8                    # j-chunks per sigma quarter-slab
ALPHA, BETA, BN_EPS = 0.005, 5e-5, 1e-5

F32 = mybir.dt.float32
BF16 = mybir.dt.bfloat16
AF = mybir.ActivationFunctionType
ALU = mybir.AluOpType
XY = mybir.AxisListType.XY

_CACHE = {}


def build_program():
    nc = bacc.Bacc("TRN2", target_bir_lowering=False, debug=False,
                   enable_asserts=False, num_devices=NC)

    def din(name, shape, dt):
        return nc.dram_tensor(name, shape, dt, kind="ExternalInput").ap()

    xT_aug = din("xT_aug", [NI + 1, B * G], F32)           # row 8 = ones
    xT_loc = din("xT_loc", [NI + 1, B * GL], F32)          # row 8 = ones
    w1T = din("w1T", [G, GL], BF16)
    w2T = din("w2T", [G, GL], BF16)
    W_aug = din("W_aug", [NI + 1, H], F32)
    We1_bf = din("We1_bf", [H + 1, 2], BF16)
    We2_bf = din("We2_bf", [H + 1, 2], BF16)
    We1_f = din("We1_f", [H + 1, 2], F32)
    We2_f = din("We2_f", [H + 1, 2], F32)
    Wn1a = din("Wn1a", [H + 1, NO], F32)
    Wn1b = din("Wn1b", [H, NO], F32)
    Wm1a = din("Wm1a", [H + 1, NO], F32)
    Wm1b = din("Wm1b", [H + 1, NO], F32)
    Wn2a = din("Wn2a", [H + 1, NO], F32)
    Wn2b = din("Wn2b", [H, NO], F32)
    Wm2a = din("Wm2a", [H + 1, NO], F32)
    Wm2b = din("Wm2b", [H + 1, NO], F32)
    bn_g_nat = din("bn_g_nat", [128, LCH], F32)
    bn_b_nat = din("bn_b_nat", [128, LCH], F32)
    bn_g_row = din("bn_g_row", [1, GL], F32)
    bn_b_row = din("bn_b_row", [1, GL], F32)

    out = nc.dram_tensor("out", [B * GL, NO], F32, kind="ExternalOutput").ap()
    out_r = out.rearrange("(b l p) f -> p b l f", b=B, l=LCH, p=128)

    with tile.TileContext(nc) as tc:
        with (
            tc.tile_pool(name="cp", bufs=1) as cp,
            tc.tile_pool(name="bp", bufs=1) as bp,
            tc.tile_pool(name="wp", bufs=1) as wp,
            tc.tile_pool(name="sp", bufs=2) as sp,
            tc.tile_pool(name="pp", bufs=1, space="PSUM") as pp,
            tc.tile_pool(name="dp", bufs=1, space="DRAM") as dp,
        ):
            # ---------- constants ----------
            W_aug_sb = cp.tile([NI + 1, H], F32, name="W_aug_sb", tag="W_aug_sb")
            nc.sync.dma_start(W_aug_sb[:], W_aug[:])
            sm = {}
            for nm, ap_ in [("We1_bf", We1_bf), ("We2_bf", We2_bf),
                            ("We1_f", We1_f), ("We2_f", We2_f),
                            ("Wn1a", Wn1a), ("Wn1b", Wn1b),
                            ("Wm1a", Wm1a), ("Wm1b", Wm1b),
                            ("Wn2a", Wn2a), ("Wn2b", Wn2b),
                            ("Wm2a", Wm2a), ("Wm2b", Wm2b),
                            ("bn_g_nat", bn_g_nat), ("bn_b_nat", bn_b_nat),
                            ("bn_g_row", bn_g_row), ("bn_b_row", bn_b_row)]:
                t = cp.tile(list(ap_.shape), ap_.dtype, name=f"{nm}_sb",
                            tag=f"{nm}_sb")
                nc.sync.dma_start(t[:], ap_[:])
                sm[nm] = t
            ones_c = cp.tile([1, 128], F32, name="ones_c", tag="ones_c")
            nc.vector.memset(ones_c[:], 1.0)
            onesk = cp.tile([H, 1], F32, name="onesk", tag="onesk")
            nc.vector.memset(onesk[:], 1.0)
            xTl_sb = cp.tile([NI + 1, B * GL], F32, name="xTl_sb", tag="xTl_sb")
            nc.sync.dma_start(xTl_sb[:], xT_loc[:])

            # ---------- big resident tensors ----------
            w1T_sb = bp.tile([128, NCH * GL], BF16, name="w1T_sb", tag="w1T_sb")
            w2T_sb = bp.tile([128, NCH * GL], BF16, name="w2T_sb", tag="w2T_sb")
            w1T_r = w1T.rearrange("(k p) i -> p k i", p=128)
            w2T_r = w2T.rearrange("(k p) i -> p k i", p=128)
            for kq in range(4):
                nc.sync.dma_start(
                    w1T_sb[:, kq * QC * GL:(kq + 1) * QC * GL],
                    w1T_r[:, kq * QC:(kq + 1) * QC])
            h0T = bp.tile([H + 1, B * G], BF16, name="h0T", tag="h0T")
            h0n = bp.tile([128, B * NCH * (H + 1)], BF16, name="h0n", tag="h0n")
            h0l = bp.tile([H + 1, B * GL], F32, name="h0l", tag="h0l")
            nodes1T = bp.tile([H + 1, B * GL], F32, name="nodes1T", tag="nodes1T")
            nodes2T = bp.tile([H + 1, B * GL], F32, name="nodes2T", tag="nodes2T")
            hbnT_f = bp.tile([H + 1, B * GL], F32, name="hbnT_f", tag="hbnT_f")
            ghat = bp.tile([128, B * NCH * (H + 1)], BF16, name="ghat", tag="ghat")
            nc.vector.memset(h0T[H:H + 1, :], 1.0)
            nc.vector.memset(h0n[:], 1.0)
            nc.vector.memset(h0l[H:H + 1, :], 1.0)
            nc.vector.memset(nodes1T[H:H + 1, :], 1.0)
            nc.vector.memset(nodes2T[H:H + 1, :], 1.0)
            nc.vector.memset(hbnT_f[H:H + 1, :], 1.0)

            def elu(z_psum, out_ap, shape):
                p, f = shape
                tf = wp.tile([128, GL], F32, name="elu_t", tag="elu_t", bufs=3)
                t1 = tf[0:p, 0:f]
                nc.vector.tensor_scalar_min(t1, z_psum, 0.0)
                nc.scalar.activation(t1, t1, AF.Exp)
                nc.vector.tensor_scalar_add(t1, t1, -1.0)
                nc.vector.tensor_tensor(out_ap, z_psum, t1, op=ALU.max)

            # ---------- phase 1: h0 in both layouts ----------
            for kq in range(8):
                xq = wp.tile([NI + 1, 8 * 128], F32, name="xq", tag="xq", bufs=2)
                nc.sync.dma_start(xq[:], xT_aug[:, kq * 1024:(kq + 1) * 1024])
                for s in range(2):
                    ps = pp.tile([H, 512], F32, name="ps_sm", tag="sm", bufs=4)
                    nc.tensor.matmul(ps[:], W_aug_sb[:],
                                     xq[:, s * 512:(s + 1) * 512],
                                     start=True, stop=True)
                    col = kq * 1024 + s * 512
                    elu(ps[:], h0T[0:H, col:col + 512], [H, 512])
                for s in range(8):
                    q = kq * 8 + s
                    ps = pp.tile([128, H], F32, name="ps_sm", tag="sm", bufs=4)
                    nc.tensor.matmul(ps[:], xq[:, s * 128:(s + 1) * 128],
                                     W_aug_sb[:], start=True, stop=True)
                    elu(ps[:], h0n[:, q * (H + 1):q * (H + 1) + H], [128, H])
            for b in range(B):
                ps = pp.tile([H, GL], F32, name="ps_sm", tag="sm", bufs=4)
                nc.tensor.matmul(ps[:], W_aug_sb[:],
                                 xTl_sb[:, b * GL:(b + 1) * GL],
                                 start=True, stop=True)
                elu(ps[:], h0l[0:H, b * GL:(b + 1) * GL], [H, GL])

            gather_nat = dp.tile([128, B * LCH * (H + 1)], BF16, name="gnat_in",
                                 tag="gnat_in")
            gather_natO = dp.tile([NC * 128, B * LCH * (H + 1)], BF16,
                                  addr_space="Shared", name="gnat_out",
                                  tag="gnat_out")
            gather_T = dp.tile([H + 1, B * GL], BF16, name="gT_in", tag="gT_in")
            gather_TO = dp.tile([NC * (H + 1), B * GL], BF16,
                                addr_space="Shared", name="gT_out", tag="gT_out")

            # ---------- one message-passing block ----------
            def mp_block(blk, wT_sb, We_bf, We_f, Wna, Wnb, Wma, Wmb,
                         hT_slice, h_nat, hTl, nodesT, merge_dst):
                ssrc = wp.tile([128, B * NCH], F32, name=f"ssrc{blk}",
                               tag=f"ssrc{blk}")
                ps_s = pp.tile([128, B * NCH], F32, name="ps_ssrc", tag="sm",
                               bufs=4)
                for q in range(B * NCH):
                    nc.tensor.matmul(ps_s[:, q:q + 1], hT_slice(q),
                                     We_bf[:, 0:1], start=True, stop=True)
                nc.vector.tensor_copy(ssrc[:], ps_s[:])
                for b in range(B):
                    ps_d = pp.tile([1, GL], F32, name="ps_d", tag="sm", bufs=4)
                    nc.tensor.matmul(ps_d[:], We_f[:, 1:2],
                                     hTl[:, b * GL:(b + 1) * GL],
                                     start=True, stop=True)
                    sd_row = wp.tile([1, GL], F32, name="sd_row", tag="sd_row",
                                     bufs=1)
                    nc.vector.tensor_copy(sd_row[:], ps_d[:])
                    ps_bc = pp.tile([128, GL], F32, name="ps_bc", tag="bc",
                                    bufs=2)
                    nc.tensor.matmul(ps_bc[:], ones_c[:], sd_row[:],
                                     start=True, stop=True)
                    sdb = wp.tile([128, GL], F32, name="sdb", tag="sdb", bufs=2)
                    nc.vector.tensor_copy(sdb[:], ps_bc[:])

                    ps_acc = pp.tile([H + 1, GL], F32, name="ps_acc", tag="acc",
                                     bufs=2)
                    for qq in range(NCH // QC):
                        sig = sp.tile([128, QC * GL], BF16, name="sig",
                                      tag="sig", bufs=2)
                        for k8 in range(QC):
                            k = qq * QC + k8
                            nc.scalar.activation(
                                sig[:, k8 * GL:(k8 + 1) * GL], sdb[:],
                                AF.Sigmoid,
                                bias=ssrc[:, b * NCH + k:b * NCH + k + 1])
                        for hh in range(QC // 4):
                            sl = slice(hh * 4 * GL, (hh + 1) * 4 * GL)
                            wsl = slice((qq * QC + hh * 4) * GL,
                                        (qq * QC + hh * 4 + 4) * GL)
                            nc.vector.tensor_tensor(sig[:, sl], sig[:, sl],
                                                    wT_sb[:, wsl], op=ALU.mult)
                        for k8 in range(QC):
                            k = qq * QC + k8
                            q = b * NCH + k
                            nc.tensor.matmul(
                                ps_acc[:],
                                h_nat[:, q * (H + 1):(q + 1) * (H + 1)],
                                sig[:, k8 * GL:(k8 + 1) * GL],
                                start=(k == 0), stop=(k == NCH - 1))
                    # nodes MLP (feature-major)
                    rsrc = wp.tile([H + 1, GL], F32, name="rsrc", tag="rsrc",
                                   bufs=2)
                    nc.vector.tensor_copy(rsrc[0:H, :], ps_acc[0:H, :])
                    nc.vector.memset(rsrc[H:H + 1, :], 1.0)
                    rs_row = wp.tile([1, GL], F32, name="rs_row", tag="rs_row",
                                     bufs=2)
                    nc.vector.tensor_copy(rs_row[:], ps_acc[H:H + 1, :])
                    ps_rb = pp.tile([H, GL], F32, name="ps_rb", tag="bc", bufs=2)
                    nc.tensor.matmul(ps_rb[:], ones_c[:, 0:H], rs_row[:],
                                     start=True, stop=True)
                    hdT = wp.tile([H, GL], F32, name="hdT", tag="hdT", bufs=2)
                    nc.vector.tensor_tensor(hdT[:],
                                            hTl[0:H, b * GL:(b + 1) * GL],
                                            ps_rb[:], op=ALU.mult)
                    ps_n = pp.tile([H, GL], F32, name="ps_n", tag="sm", bufs=4)
                    nc.tensor.matmul(ps_n[:], Wna[:], rsrc[:],
                                     start=True, stop=False)
                    nc.tensor.matmul(ps_n[:], Wnb[:], hdT[:],
                                     start=False, stop=True)
                    elu(ps_n[:], nodesT[0:H, b * GL:(b + 1) * GL], [H, GL])
                    for l in range(LCH):
                        ps_m = pp.tile([128, NO], F32, name="ps_m", tag="sm",
                                       bufs=4)
                        c0 = b * GL + l * 128
                        nc.tensor.matmul(ps_m[:], nodesT[:, c0:c0 + 128],
                                         Wma[:], start=True, stop=False)
                        nc.tensor.matmul(ps_m[:], hTl[:, c0:c0 + 128],
                                         Wmb[:], start=False, stop=True)
                        merge_dst(b, l, ps_m)

            # ---------- block 1 ----------
            h1n = wp.tile([128, B * LCH * NO], F32, name="h1n", tag="h1n")

            def merge1_dst(b, l, ps_m):
                q = b * LCH + l
                elu(ps_m[:], h1n[:, q * NO:(q + 1) * NO], [128, NO])

            mp_block(1, w1T_sb, sm["We1_bf"], sm["We1_f"],
                     sm["Wn1a"], sm["Wn1b"], sm["Wm1a"], sm["Wm1b"],
                     lambda q: h0T[:, q * 128:(q + 1) * 128],
                     h0n, h0l, nodes1T, merge1_dst)
            for kq in range(4):
                nc.sync.dma_start(
                    w2T_sb[:, kq * QC * GL:(kq + 1) * QC * GL],
                    w2T_r[:, kq * QC:(kq + 1) * QC])

            h1T = wp.tile([H, B * GL], F32, name="h1T", tag="h1T")
            for b in range(B):
                ps = pp.tile([H, GL], F32, name="ps_sm2", tag="sm", bufs=4)
                nc.tensor.matmul(ps[:], sm["Wm1a"][:],
                                 nodes1T[:, b * GL:(b + 1) * GL],
                                 start=True, stop=False)
                nc.tensor.matmul(ps[:], sm["Wm1b"][:],
                                 h0l[:, b * GL:(b + 1) * GL],
                                 start=False, stop=True)
                elu(ps[:], h1T[:, b * GL:(b + 1) * GL], [H, GL])

            # ---------- BatchNorm (fully local) ----------
            stat = wp.tile([128, 6 * LCH], F32, name="stat", tag="stat")
            mu_n, var_n = stat[:, 0:LCH], stat[:, LCH:2 * LCH]
            scl_n, shf_n = stat[:, 2 * LCH:3 * LCH], stat[:, 3 * LCH:4 * LCH]
            t_n, t2_n = stat[:, 4 * LCH:5 * LCH], stat[:, 5 * LCH:6 * LCH]
            sq_n = wp.tile([128, B * LCH * NO], F32, name="sq_n", tag="sq_n")
            nc.scalar.activation(sq_n[:], h1n[:], AF.Square)
            h1n_r = h1n.rearrange("p (b l f) -> p b l f", b=B, l=LCH)
            sq_r = sq_n.rearrange("p (b l f) -> p b l f", b=B, l=LCH)
            for l in range(LCH):
                nc.vector.reduce_sum(mu_n[:, l:l + 1], h1n_r[:, :, l, :], axis=XY)
                nc.vector.reduce_sum(var_n[:, l:l + 1], sq_r[:, :, l, :], axis=XY)
            nc.vector.tensor_scalar_mul(mu_n, mu_n, 1.0 / (B * NO))
            nc.vector.tensor_scalar_mul(var_n, var_n, 1.0 / (B * NO))
            nc.vector.tensor_tensor(t_n, mu_n, mu_n, op=ALU.mult)
            nc.vector.tensor_tensor(var_n, var_n, t_n, op=ALU.subtract)
            nc.vector.tensor_scalar_add(t_n, var_n, BN_EPS)
            nc.scalar.activation(t_n, t_n, AF.Ln)
            nc.scalar.activation(t_n, t_n, AF.Exp, scale=-0.5)
            nc.vector.tensor_tensor(scl_n, t_n, sm["bn_g_nat"][:], op=ALU.mult)
            nc.vector.tensor_tensor(t2_n, mu_n, scl_n, op=ALU.mult)
            nc.vector.tensor_tensor(shf_n, sm["bn_b_nat"][:], t2_n,
                                    op=ALU.subtract)
            hbn_n = wp.tile([128, B * LCH * (H + 1)], BF16, name="hbn_n",
                            tag="hbn_n")
            nc.vector.memset(hbn_n[:], 1.0)
            for b in range(B):
                for l in range(LCH):
                    q = b * LCH + l
                    nc.vector.tensor_scalar(
                        hbn_n[:, q * (H + 1):q * (H + 1) + H],
                        h1n[:, q * NO:(q + 1) * NO],
                        scl_n[:, l:l + 1], shf_n[:, l:l + 1],
                        op0=ALU.mult, op1=ALU.add)

            # row-layout stats for the feature-major copy
            rowb = wp.tile([1, 4 * GL], F32, name="rowb", tag="rowb")
            mu_r, var_r = rowb[:, 0:GL], rowb[:, GL:2 * GL]
            scl_r, shf_r = rowb[:, 2 * GL:3 * GL], rowb[:, 3 * GL:4 * GL]
            t_r, t2_r = scl_r, shf_r
            sqT = wp.tile([H, B * GL], F32, name="sqT", tag="sqT")
            nc.scalar.activation(sqT[:], h1T[:], AF.Square)
            ps_r0 = pp.tile([1, GL], F32, name="ps_r0", tag="sm", bufs=4)
            for b in range(B):
                nc.tensor.matmul(ps_r0[:], onesk[:],
                                 h1T[:, b * GL:(b + 1) * GL],
                                 start=(b == 0), stop=(b == B - 1))
            ps_r1 = pp.tile([1, GL], F32, name="ps_r1", tag="sm", bufs=4)
            for b in range(B):
                nc.tensor.matmul(ps_r1[:], onesk[:],
                                 sqT[:, b * GL:(b + 1) * GL],
                                 start=(b == 0), stop=(b == B - 1))
            nc.vector.tensor_scalar_mul(mu_r, ps_r0[:], 1.0 / (B * NO))
            nc.vector.tensor_scalar_mul(var_r, ps_r1[:], 1.0 / (B * NO))
            nc.vector.tensor_tensor(t_r, mu_r, mu_r, op=ALU.mult)
            nc.vector.tensor_tensor(var_r, var_r, t_r, op=ALU.subtract)
            nc.vector.tensor_scalar_add(t_r, var_r, BN_EPS)
            nc.scalar.activation(t_r, t_r, AF.Ln)
            nc.scalar.activation(t_r, t_r, AF.Exp, scale=-0.5)
            nc.vector.tensor_tensor(scl_r, t_r, sm["bn_g_row"][:], op=ALU.mult)
            nc.vector.tensor_tensor(t2_r, mu_r, scl_r, op=ALU.mult)
            nc.vector.tensor_tensor(shf_r, sm["bn_b_row"][:], t2_r,
                                    op=ALU.subtract)
            ps_sc = pp.tile([H, GL], F32, name="ps_sc", tag="bc", bufs=2)
            nc.tensor.matmul(ps_sc[:], ones_c[:, 0:H], scl_r, start=True,
                             stop=True)
            ps_sh = pp.tile([H, GL], F32, name="ps_sh", tag="bc", bufs=2)
            nc.tensor.matmul(ps_sh[:], ones_c[:, 0:H], shf_r, start=True,
                             stop=True)
            for b in range(B):
                sl = slice(b * GL, (b + 1) * GL)
                nc.vector.tensor_tensor(hbnT_f[0:H, sl], h1T[:, sl], ps_sc[:],
                                        op=ALU.mult)
                nc.vector.tensor_tensor(hbnT_f[0:H, sl], hbnT_f[0:H, sl],
                                        ps_sh[:], op=ALU.add)
            hbnT_bf = wp.tile([H + 1, B * GL], BF16, name="hbnT_bf",
                              tag="hbnT_bf")
            nc.vector.memset(hbnT_bf[H:H + 1, :], 1.0)
            nc.vector.tensor_copy(hbnT_bf[0:H, :], hbnT_f[0:H, :])

            # ---------- AllGather both layouts ----------
            nc.sync.dma_start(gather_nat[:], hbn_n[:])
            nc.sync.dma_start(gather_T[:], hbnT_bf[:])
            nc.gpsimd.collective_compute(
                "AllGather", ALU.bypass, replica_groups=[list(range(NC))],
                ins=[gather_nat.opt()], outs=[gather_natO.opt()])
            nc.gpsimd.collective_compute(
                "AllGather", ALU.bypass, replica_groups=[list(range(NC))],
                ins=[gather_T.opt()], outs=[gather_TO.opt()])
            W1 = H + 1
            for c in range(NC):
                for b in range(B):
                    nc.sync.dma_start(
                        ghat[:, (b * NCH + c * LCH) * W1:
                             (b * NCH + c * LCH + LCH) * W1],
                        gather_natO[c * 128:(c + 1) * 128,
                                    b * LCH * W1:(b + 1) * LCH * W1])

            def gT_slice(q):
                b, kk = q // NCH, q % NCH
                c, l = kk // LCH, kk % LCH
                t = wp.tile([W1, 128], BF16, name="gTs", tag="gTs", bufs=4)
                nc.sync.dma_start(
                    t[:], gather_TO[c * W1:(c + 1) * W1,
                                    b * GL + l * 128:b * GL + (l + 1) * 128])
                return t[:]

            # ---------- block 2 ----------
            out_n = wp.tile([128, B * LCH * NO], F32, name="out_n", tag="out_n")

            def merge2_dst(b, l, ps_m):
                q = b * LCH + l
                elu(ps_m[:], out_n[:, q * NO:(q + 1) * NO], [128, NO])

            mp_block(2, w2T_sb, sm["We2_bf"], sm["We2_f"],
                     sm["Wn2a"], sm["Wn2b"], sm["Wm2a"], sm["Wm2b"],
                     gT_slice, ghat, hbnT_f, nodes2T, merge2_dst)

            nc.sync.dma_start(out_r, out_n[:])

    nc.compile()
    return nc


def _prep_inputs(x, edges1, edges2, W_infer, b_infer, W_e1, b_e1, W_e2, b_e2,
                 W_n1, b_n1, W_n2, b_n2, W_m1, b_m1, W_m2, b_m2,
                 bn_gamma, bn_beta):
    f32 = np.float32
    bf16 = ml_dtypes.bfloat16
    xT = np.asarray(x, f32).transpose(2, 0, 1).reshape(NI, B * G)
    xT_aug = np.concatenate([xT, np.ones((1, B * G), f32)], axis=0)
    w1 = (ALPHA + (1.0 - ALPHA) * np.asarray(edges1, f32)).astype(bf16)
    w2 = (BETA + (1.0 - BETA) * np.asarray(edges2, f32)).astype(bf16)

    def wecat(W_e, b_e):
        c0 = np.concatenate([np.asarray(W_e, f32)[:H, 0], [0.0]]).astype(f32)
        c1 = np.concatenate([np.asarray(W_e, f32)[H:, 0],
                             [np.asarray(b_e, f32)[0]]]).astype(f32)
        return np.stack([c0, c1], axis=1)

    We1 = wecat(W_e1, b_e1)
    We2 = wecat(W_e2, b_e2)

    def vcat(W, b_):
        return np.concatenate([np.asarray(W, f32),
                               np.asarray(b_, f32)[None, :]], 0)

    com = dict(
        xT_aug=xT_aug, W_aug=vcat(W_infer, b_infer),
        We1_bf=We1.astype(bf16), We2_bf=We2.astype(bf16),
        We1_f=We1, We2_f=We2,
        Wn1a=vcat(np.asarray(W_n1, f32)[:H], b_n1),
        Wn1b=np.asarray(W_n1, f32)[H:],
        Wm1a=vcat(np.asarray(W_m1, f32)[:H], b_m1),
        Wm1b=vcat(np.asarray(W_m1, f32)[H:], np.zeros(NO, f32)),
        Wn2a=vcat(np.asarray(W_n2, f32)[:H], b_n2),
        Wn2b=np.asarray(W_n2, f32)[H:],
        Wm2a=vcat(np.asarray(W_m2, f32)[:H], b_m2),
        Wm2b=vcat(np.asarray(W_m2, f32)[H:], np.zeros(NO, f32)),
    )
    in_maps = []
    for c in range(NC):
        sl = slice(c * GL, (c + 1) * GL)
        xl = np.asarray(x, f32)[:, sl, :].transpose(2, 0, 1).reshape(NI, B * GL)
        m = dict(com)
        m["xT_loc"] = np.concatenate([xl, np.ones((1, B * GL), f32)], 0)
        m["w1T"] = np.ascontiguousarray(w1[sl, :].T)
        m["w2T"] = np.ascontiguousarray(w2[sl, :].T)
        g = np.asarray(bn_gamma, f32)[sl]
        b_ = np.asarray(bn_beta, f32)[sl]
        m["bn_g_nat"] = np.ascontiguousarray(g.reshape(LCH, 128).T)
        m["bn_b_nat"] = np.ascontiguousarray(b_.reshape(LCH, 128).T)
        m["bn_g_row"] = np.ascontiguousarray(g[None, :])
        m["bn_b_row"] = np.ascontiguousarray(b_[None, :])
        in_maps.append(m)
    return in_maps


def kernel(**inputs):
    if "nc" not in _CACHE:
        _CACHE["nc"] = build_program()
    nc = _CACHE["nc"]
    in_maps = _prep_inputs(**inputs)
    res = run_bass_kernel_spmd(nc, in_maps, list(range(NC)))
    parts = [res.results[c]["out"].reshape(B, GL, NO) for c in range(NC)]
    return np.concatenate(parts, axis=1).astype(np.float32)


# revision 8
# speedup vs baseline: 1.4676x; 1.4676x over previous
"""Trainium2 Bass kernel for gnn_message_passing (nn_BFR_28089086116615).

Sharding: receiver axis i (G=4096 -> 8 cores x 512). Host pre-transposes the
edge matrices and folds the {coef, 1} gate weights in bf16: wT[j, i]. On
device, sigma^T is computed natively in [j-partition, i-free] layout (ACT
sigmoid, per-partition bias = s_src[j-chunk], input = broadcast s_dst row),
gated by wT on DVE (bf16 2x), and contracted on PE with stationary weights
[h | 1] so the receiver rowsum rides along as an extra psum row. BatchNorm is
per-gene -> fully local; one AllGather (both layouts) of normalized h between
the two message-passing blocks.
"""
import sys
sys.path.insert(0, "/opt/trn_rl_repo")
import numpy as np
import ml_dtypes

import concourse.bass as bass
import concourse.bacc as bacc
import concourse.mybir as mybir
import concourse.tile as tile
from concourse.bass_utils import run_bass_kernel_spmd

NC = 8
B, G, NI, H, NO = 2, 4096, 8, 32, 32
GL = G // NC              # 512 local receivers per core
LCH = GL // 128           # 4 local chunks
NCH = G // 128            # 32 global j-chunks
QC = 8                    # j-chunks per sigma quarter-slab
ALPHA, BETA, BN_EPS = 0.005, 5e-5, 1e-5

F32 = mybir.dt.float32
BF16 = mybir.dt.bfloat16
AF = mybir.ActivationFunctionType
ALU = mybir.AluOpType
XY = mybir.AxisListType.XY

_CACHE = {}


def build_program():
    nc = bacc.Bacc("TRN2", target_bir_lowering=False, debug=False,
                   enable_asserts=False, num_devices=NC)

    def din(name, shape, dt):
        return nc.dram_tensor(name, shape, dt, kind="ExternalInput").ap()

    xT_aug = din("xT_aug", [NI + 1, B * G], F32)           # row 8 = ones
    xT_loc = din("xT_loc", [NI + 1, B * GL], F32)          # row 8 = ones
    w1T = din("w1T", [G, GL], BF16)
    w2T = din("w2T", [G, GL], BF16)
    W_aug = din("W_aug", [NI + 1, H], F32)
    We1_f = din("We1_f", [H + 1, 2], F32)
    We2_f = din("We2_f", [H + 1, 2], F32)
    We1_rep = din("We1_rep", [1, B * NCH * H], F32)
    We2_rep = din("We2_rep", [1, B * NCH * H], F32)
    Wn1a = din("Wn1a", [H + 1, NO], F32)
    Wn1b = din("Wn1b", [H, NO], F32)
    Wm1a = din("Wm1a", [H + 1, NO], F32)
    Wm1b = din("Wm1b", [H + 1, NO], F32)
    Wn2a = din("Wn2a", [H + 1, NO], F32)
    Wn2b = din("Wn2b", [H, NO], F32)
    Wm2a = din("Wm2a", [H + 1, NO], F32)
    Wm2b = din("Wm2b", [H + 1, NO], F32)
    bn_g_nat = din("bn_g_nat", [128, LCH], F32)
    bn_b_nat = din("bn_b_nat", [128, LCH], F32)
    bn_g_row = din("bn_g_row", [1, GL], F32)
    bn_b_row = din("bn_b_row", [1, GL], F32)

    out = nc.dram_tensor("out", [B * GL, NO], F32, kind="ExternalOutput").ap()
    out_r = out.rearrange("(b l p) f -> p b l f", b=B, l=LCH, p=128)

    with tile.TileContext(nc) as tc:
        with (
            tc.tile_pool(name="cp", bufs=1) as cp,
            tc.tile_pool(name="bp", bufs=1) as bp,
            tc.tile_pool(name="wp", bufs=1) as wp,
            tc.tile_pool(name="sp", bufs=2) as sp,
            tc.tile_pool(name="pp", bufs=1, space="PSUM") as pp,
            tc.tile_pool(name="dp", bufs=1, space="DRAM") as dp,
        ):
            # ---------- constants ----------
            W_aug_sb = cp.tile([NI + 1, H], F32, name="W_aug_sb", tag="W_aug_sb")
            nc.sync.dma_start(W_aug_sb[:], W_aug[:])
            sm = {}
            for nm, ap_ in [("We1_rep", We1_rep), ("We2_rep", We2_rep),
                            ("We1_f", We1_f), ("We2_f", We2_f),
                            ("Wn1a", Wn1a), ("Wn1b", Wn1b),
                            ("Wm1a", Wm1a), ("Wm1b", Wm1b),
                            ("Wn2a", Wn2a), ("Wn2b", Wn2b),
                            ("Wm2a", Wm2a), ("Wm2b", Wm2b),
                            ("bn_g_nat", bn_g_nat), ("bn_b_nat", bn_b_nat),
                            ("bn_g_row", bn_g_row), ("bn_b_row", bn_b_row)]:
                t = cp.tile(list(ap_.shape), ap_.dtype, name=f"{nm}_sb",
                            tag=f"{nm}_sb")
                nc.sync.dma_start(t[:], ap_[:])
                sm[nm] = t
            ones_c = cp.tile([1, 128], F32, name="ones_c", tag="ones_c")
            nc.vector.memset(ones_c[:], 1.0)
            onesk = cp.tile([H, 1], F32, name="onesk", tag="onesk")
            nc.vector.memset(onesk[:], 1.0)
            xTl_sb = cp.tile([NI + 1, B * GL], F32, name="xTl_sb", tag="xTl_sb")
            nc.sync.dma_start(xTl_sb[:], xT_loc[:])

            # ---------- big resident tensors ----------
            w1T_sb = bp.tile([128, NCH * GL], BF16, name="w1T_sb", tag="w1T_sb")
            w2T_sb = bp.tile([128, NCH * GL], BF16, name="w2T_sb", tag="w2T_sb")
            w1T_r = w1T.rearrange("(k p) i -> p k i", p=128)
            w2T_r = w2T.rearrange("(k p) i -> p k i", p=128)
            for kq in range(4):
                nc.sync.dma_start(
                    w1T_sb[:, kq * QC * GL:(kq + 1) * QC * GL],
                    w1T_r[:, kq * QC:(kq + 1) * QC])
            h0n = bp.tile([128, B * NCH * (H + 1)], BF16, name="h0n", tag="h0n")
            h0l = bp.tile([H + 1, B * GL], F32, name="h0l", tag="h0l")
            nodes1T = bp.tile([H + 1, B * GL], F32, name="nodes1T", tag="nodes1T")
            nodes2T = bp.tile([H + 1, B * GL], F32, name="nodes2T", tag="nodes2T")
            hbnT_f = bp.tile([H + 1, B * GL], F32, name="hbnT_f", tag="hbnT_f")
            ghat = bp.tile([128, B * NCH * (H + 1)], BF16, name="ghat", tag="ghat")
            nc.vector.memset(h0n[:], 1.0)
            nc.vector.memset(h0l[H:H + 1, :], 1.0)
            nc.vector.memset(nodes1T[H:H + 1, :], 1.0)
            nc.vector.memset(nodes2T[H:H + 1, :], 1.0)
            nc.vector.memset(hbnT_f[H:H + 1, :], 1.0)

            def elu(z_psum, out_ap, shape):
                p, f = shape
                tf = wp.tile([128, GL], F32, name="elu_t", tag="elu_t", bufs=3)
                t1 = tf[0:p, 0:f]
                nc.vector.tensor_scalar_min(t1, z_psum, 0.0)
                nc.scalar.activation(t1, t1, AF.Exp)
                nc.vector.tensor_scalar_add(t1, t1, -1.0)
                nc.vector.tensor_tensor(out_ap, z_psum, t1, op=ALU.max)

            # ---------- phase 1: h0 in both layouts ----------
            for kq in range(8):
                xq = wp.tile([NI + 1, 8 * 128], F32, name="xq", tag="xq", bufs=2)
                nc.sync.dma_start(xq[:], xT_aug[:, kq * 1024:(kq + 1) * 1024])
                ps = pp.tile([128, 8 * H], F32, name="ps_sm", tag="sm", bufs=4)
                for s in range(8):
                    nc.tensor.matmul(ps[:, s * H:(s + 1) * H],
                                     xq[:, s * 128:(s + 1) * 128],
                                     W_aug_sb[:], start=True, stop=True)
                h0n_v = h0n.rearrange("p (q e) -> p q e", e=H + 1)
                elu(ps[:], h0n_v[:, kq * 8:(kq + 1) * 8, 0:H], [128, 8 * H])
            for b in range(B):
                ps = pp.tile([H, GL], F32, name="ps_sm", tag="sm", bufs=4)
                nc.tensor.matmul(ps[:], W_aug_sb[:],
                                 xTl_sb[:, b * GL:(b + 1) * GL],
                                 start=True, stop=True)
                elu(ps[:], h0l[0:H, b * GL:(b + 1) * GL], [H, GL])

            gather_nat = dp.tile([128, B * LCH * (H + 1)], BF16, name="gnat_in",
                                 tag="gnat_in")
            gather_natO = dp.tile([NC * 128, B * LCH * (H + 1)], BF16,
                                  addr_space="Shared", name="gnat_out",
                                  tag="gnat_out")

            # ---------- one message-passing block ----------
            def mp_block(blk, wT_sb, We_rep, We_f, Wna, Wnb, Wma, Wmb,
                         h_nat, hTl, nodesT, merge_dst):
                # s_src[p, q] = sum_f h_nat[p, q*33+f] * We_src[f]  (DVE)
                wrep = wp.tile([128, B * NCH * H], F32, name="wrep", tag="wrep",
                               bufs=1)
                for c4 in range(B * NCH * H // 512):
                    ps_w = pp.tile([128, 512], F32, name="ps_w", tag="bc",
                                   bufs=2)
                    nc.tensor.matmul(ps_w[:], ones_c[:],
                                     We_rep[:, c4 * 512:(c4 + 1) * 512],
                                     start=True, stop=True)
                    nc.vector.tensor_copy(wrep[:, c4 * 512:(c4 + 1) * 512],
                                          ps_w[:])
                ssx = wp.tile([128, B * NCH * H], F32, name="ssx", tag="ssx",
                              bufs=1)
                h_nat_v = h_nat.rearrange("p (q e) -> p q e", e=H + 1)
                wrep_v = wrep.rearrange("p (q f) -> p q f", f=H)
                ssx_v = ssx.rearrange("p (q f) -> p q f", f=H)
                nc.vector.tensor_tensor(ssx_v, h_nat_v[:, :, 0:H], wrep_v,
                                        op=ALU.mult)
                ssrc = wp.tile([128, B * NCH], F32, name=f"ssrc{blk}",
                               tag=f"ssrc{blk}")
                nc.vector.reduce_sum(ssrc[:], ssx_v, axis=mybir.AxisListType.X)
                accs = []
                for b in range(B):
                    ps_d = pp.tile([1, GL], F32, name="ps_d", tag="sm", bufs=4)
                    nc.tensor.matmul(ps_d[:], We_f[:, 1:2],
                                     hTl[:, b * GL:(b + 1) * GL],
                                     start=True, stop=True)
                    sd_row = wp.tile([1, GL], F32, name="sd_row", tag="sd_row",
                                     bufs=1)
                    nc.vector.tensor_copy(sd_row[:], ps_d[:])
                    ps_bc = pp.tile([128, GL], F32, name="ps_bc", tag="bc",
                                    bufs=2)
                    nc.tensor.matmul(ps_bc[:], ones_c[:], sd_row[:],
                                     start=True, stop=True)
                    sdb = wp.tile([128, GL], F32, name="sdb", tag="sdb", bufs=2)
                    nc.vector.tensor_copy(sdb[:], ps_bc[:])

                    ps_acc = pp.tile([H + 1, GL], F32, name="ps_acc", tag="acc",
                                     bufs=2)
                    for qq in range(NCH // QC):
                        sig = sp.tile([128, QC * GL], BF16, name="sig",
                                      tag="sig", bufs=2)
                        for k8 in range(QC):
                            k = qq * QC + k8
                            nc.scalar.activation(
                                sig[:, k8 * GL:(k8 + 1) * GL], sdb[:],
                                AF.Sigmoid,
                                bias=ssrc[:, b * NCH + k:b * NCH + k + 1])
                        for hh in range(QC // 4):
                            sl = slice(hh * 4 * GL, (hh + 1) * 4 * GL)
                            wsl = slice((qq * QC + hh * 4) * GL,
                                        (qq * QC + hh * 4 + 4) * GL)
                            nc.vector.tensor_tensor(sig[:, sl], sig[:, sl],
                                                    wT_sb[:, wsl], op=ALU.mult)
                        for k8 in range(QC):
                            k = qq * QC + k8
                            q = b * NCH + k
                            nc.tensor.matmul(
                                ps_acc[:],
                                h_nat[:, q * (H + 1):(q + 1) * (H + 1)],
                                sig[:, k8 * GL:(k8 + 1) * GL],
                                start=(k == 0), stop=(k == NCH - 1))
                    accs.append(ps_acc)
                for b in range(B):
                    ps_acc = accs[b]
                    # nodes MLP (feature-major)
                    rsrc = wp.tile([H + 1, GL], F32, name="rsrc", tag="rsrc",
                                   bufs=2)
                    nc.vector.tensor_copy(rsrc[0:H, :], ps_acc[0:H, :])
                    nc.vector.memset(rsrc[H:H + 1, :], 1.0)
                    rs_row = wp.tile([1, GL], F32, name="rs_row", tag="rs_row",
                                     bufs=2)
                    nc.vector.tensor_copy(rs_row[:], ps_acc[H:H + 1, :])
                    ps_rb = pp.tile([H, GL], F32, name="ps_rb", tag="bc", bufs=2)
                    nc.tensor.matmul(ps_rb[:], ones_c[:, 0:H], rs_row[:],
                                     start=True, stop=True)
                    hdT = wp.tile([H, GL], F32, name="hdT", tag="hdT", bufs=2)
                    nc.vector.tensor_tensor(hdT[:],
                                            hTl[0:H, b * GL:(b + 1) * GL],
                                            ps_rb[:], op=ALU.mult)
                    ps_n = pp.tile([H, GL], F32, name="ps_n", tag="sm", bufs=4)
                    nc.tensor.matmul(ps_n[:], Wna[:], rsrc[:],
                                     start=True, stop=False)
                    nc.tensor.matmul(ps_n[:], Wnb[:], hdT[:],
                                     start=False, stop=True)
                    elu(ps_n[:], nodesT[0:H, b * GL:(b + 1) * GL], [H, GL])
                    ps_m = pp.tile([128, LCH * NO], F32, name="ps_m", tag="sm",
                                   bufs=4)
                    for l in range(LCH):
                        c0 = b * GL + l * 128
                        nc.tensor.matmul(ps_m[:, l * NO:(l + 1) * NO],
                                         nodesT[:, c0:c0 + 128],
                                         Wma[:], start=True, stop=False)
                        nc.tensor.matmul(ps_m[:, l * NO:(l + 1) * NO],
                                         hTl[:, c0:c0 + 128],
                                         Wmb[:], start=False, stop=True)
                    merge_dst(b, ps_m)

            # ---------- block 1 ----------
            h1n = wp.tile([128, B * LCH * NO], F32, name="h1n", tag="h1n")

            def merge1_dst(b, ps_m):
                c0 = b * LCH * NO
                elu(ps_m[:], h1n[:, c0:c0 + LCH * NO], [128, LCH * NO])

            mp_block(1, w1T_sb, sm["We1_rep"], sm["We1_f"],
                     sm["Wn1a"], sm["Wn1b"], sm["Wm1a"], sm["Wm1b"],
                     h0n, h0l, nodes1T, merge1_dst)
            for kq in range(4):
                nc.sync.dma_start(
                    w2T_sb[:, kq * QC * GL:(kq + 1) * QC * GL],
                    w2T_r[:, kq * QC:(kq + 1) * QC])

            h1T = wp.tile([H, B * GL], F32, name="h1T", tag="h1T")
            for b in range(B):
                ps = pp.tile([H, GL], F32, name="ps_sm2", tag="sm", bufs=4)
                nc.tensor.matmul(ps[:], sm["Wm1a"][:],
                                 nodes1T[:, b * GL:(b + 1) * GL],
                                 start=True, stop=False)
                nc.tensor.matmul(ps[:], sm["Wm1b"][:],
                                 h0l[:, b * GL:(b + 1) * GL],
                                 start=False, stop=True)
                elu(ps[:], h1T[:, b * GL:(b + 1) * GL], [H, GL])

            # ---------- BatchNorm (fully local) ----------
            stat = wp.tile([128, 6 * LCH], F32, name="stat", tag="stat")
            mu_n, var_n = stat[:, 0:LCH], stat[:, LCH:2 * LCH]
            scl_n, shf_n = stat[:, 2 * LCH:3 * LCH], stat[:, 3 * LCH:4 * LCH]
            t_n, t2_n = stat[:, 4 * LCH:5 * LCH], stat[:, 5 * LCH:6 * LCH]
            sq_n = wp.tile([128, B * LCH * NO], F32, name="sq_n", tag="sq_n")
            nc.scalar.activation(sq_n[:], h1n[:], AF.Square)
            h1n_r = h1n.rearrange("p (b l f) -> p b l f", b=B, l=LCH)
            sq_r = sq_n.rearrange("p (b l f) -> p b l f", b=B, l=LCH)
            for l in range(LCH):
                nc.vector.reduce_sum(mu_n[:, l:l + 1], h1n_r[:, :, l, :], axis=XY)
                nc.vector.reduce_sum(var_n[:, l:l + 1], sq_r[:, :, l, :], axis=XY)
            nc.vector.tensor_scalar_mul(mu_n, mu_n, 1.0 / (B * NO))
            nc.vector.tensor_scalar_mul(var_n, var_n, 1.0 / (B * NO))
            nc.vector.tensor_tensor(t_n, mu_n, mu_n, op=ALU.mult)
            nc.vector.tensor_tensor(var_n, var_n, t_n, op=ALU.subtract)
            nc.vector.tensor_scalar_add(t_n, var_n, BN_EPS)
            nc.scalar.activation(t_n, t_n, AF.Ln)
            nc.scalar.activation(t_n, t_n, AF.Exp, scale=-0.5)
            nc.vector.tensor_tensor(scl_n, t_n, sm["bn_g_nat"][:], op=ALU.mult)
            nc.vector.tensor_tensor(t2_n, mu_n, scl_n, op=ALU.mult)
            nc.vector.tensor_tensor(shf_n, sm["bn_b_nat"][:], t2_n,
                                    op=ALU.subtract)
            hbn_n = wp.tile([128, B * LCH * (H + 1)], BF16, name="hbn_n",
                            tag="hbn_n")
            nc.vector.memset(hbn_n[:], 1.0)
            for b in range(B):
                for l in range(LCH):
                    q = b * LCH + l
                    nc.vector.tensor_scalar(
                        hbn_n[:, q * (H + 1):q * (H + 1) + H],
                        h1n[:, q * NO:(q + 1) * NO],
                        scl_n[:, l:l + 1], shf_n[:, l:l + 1],
                        op0=ALU.mult, op1=ALU.add)

            # row-layout stats for the feature-major copy
            rowb = wp.tile([1, 4 * GL], F32, name="rowb", tag="rowb")
            mu_r, var_r = rowb[:, 0:GL], rowb[:, GL:2 * GL]
            scl_r, shf_r = rowb[:, 2 * GL:3 * GL], rowb[:, 3 * GL:4 * GL]
            t_r, t2_r = scl_r, shf_r
            sqT = wp.tile([H, B * GL], F32, name="sqT", tag="sqT")
            nc.scalar.activation(sqT[:], h1T[:], AF.Square)
            ps_r0 = pp.tile([1, GL], F32, name="ps_r0", tag="sm", bufs=4)
            for b in range(B):
                nc.tensor.matmul(ps_r0[:], onesk[:],
                                 h1T[:, b * GL:(b + 1) * GL],
                                 start=(b == 0), stop=(b == B - 1))
            ps_r1 = pp.tile([1, GL], F32, name="ps_r1", tag="sm", bufs=4)
            for b in range(B):
                nc.tensor.matmul(ps_r1[:], onesk[:],
                                 sqT[:, b * GL:(b + 1) * GL],
                                 start=(b == 0), stop=(b == B - 1))
            nc.vector.tensor_scalar_mul(mu_r, ps_r0[:], 1.0 / (B * NO))
            nc.vector.tensor_scalar_mul(var_r, ps_r1[:], 1.0 / (B * NO))
            nc.vector.tensor_tensor(t_r, mu_r, mu_r, op=ALU.mult)
            nc.vector.tensor_tensor(var_r, var_r, t_r, op=ALU.subtract)
            nc.vector.tensor_scalar_add(t_r, var_r, BN_EPS)
            nc.scalar.activation(t_r, t_r, AF.Ln)
            nc.scalar.activation(t_r, t_r, AF.Exp, scale=-0.5)
            nc.vector.tensor_tensor(scl_r, t_r, sm["bn_g_row"][:], op=ALU.mult)
            nc.vector.tensor_tensor(t2_r, mu_r, scl_r, op=ALU.mult)
            nc.vector.tensor_tensor(shf_r, sm["bn_b_row"][:], t2_r,
                                    op=ALU.subtract)
            ps_sc = pp.tile([H, GL], F32, name="ps_sc", tag="bc", bufs=2)
            nc.tensor.matmul(ps_sc[:], ones_c[:, 0:H], scl_r, start=True,
                             stop=True)
            ps_sh = pp.tile([H, GL], F32, name="ps_sh", tag="bc", bufs=2)
            nc.tensor.matmul(ps_sh[:], ones_c[:, 0:H], shf_r, start=True,
                             stop=True)
            for b in range(B):
                sl = slice(b * GL, (b + 1) * GL)
                nc.vector.tensor_tensor(hbnT_f[0:H, sl], h1T[:, sl], ps_sc[:],
                                        op=ALU.mult)
                nc.vector.tensor_tensor(hbnT_f[0:H, sl], hbnT_f[0:H, sl],
                                        ps_sh[:], op=ALU.add)

            # ---------- AllGather both layouts ----------
            nc.sync.dma_start(gather_nat[:], hbn_n[:])
            nc.gpsimd.collective_compute(
                "AllGather", ALU.bypass, replica_groups=[list(range(NC))],
                ins=[gather_nat.opt()], outs=[gather_natO.opt()])
            W1 = H + 1
            for c in range(NC):
                for b in range(B):
                    nc.sync.dma_start(
                        ghat[:, (b * NCH + c * LCH) * W1:
                             (b * NCH + c * LCH + LCH) * W1],
                        gather_natO[c * 128:(c + 1) * 128,
                                    b * LCH * W1:(b + 1) * LCH * W1])


            # ---------- block 2 ----------
            out_n = wp.tile([128, B * LCH * NO], F32, name="out_n", tag="out_n")

            def merge2_dst(b, ps_m):
                c0 = b * LCH * NO
                elu(ps_m[:], out_n[:, c0:c0 + LCH * NO], [128, LCH * NO])

            mp_block(2, w2T_sb, sm["We2_rep"], sm["We2_f"],
                     sm["Wn2a"], sm["Wn2b"], sm["Wm2a"], sm["Wm2b"],
                     ghat, hbnT_f, nodes2T, merge2_dst)

            nc.sync.dma_start(out_r, out_n[:])

    nc.compile()
    return nc


def _prep_inputs(x, edges1, edges2, W_infer, b_infer, W_e1, b_e1, W_e2, b_e2,
                 W_n1, b_n1, W_n2, b_n2, W_m1, b_m1, W_m2, b_m2,
                 bn_gamma, bn_beta):
    f32 = np.float32
    bf16 = ml_dtypes.bfloat16
    xT = np.asarray(x, f32).transpose(2, 0, 1).reshape(NI, B * G)
    xT_aug = np.concatenate([xT, np.ones((1, B * G), f32)], axis=0)
    w1 = (ALPHA + (1.0 - ALPHA) * np.asarray(edges1, f32)).astype(bf16)
    w2 = (BETA + (1.0 - BETA) * np.asarray(edges2, f32)).astype(bf16)

    def wecat(W_e, b_e):
        c0 = np.concatenate([np.asarray(W_e, f32)[:H, 0], [0.0]]).astype(f32)
        c1 = np.concatenate([np.asarray(W_e, f32)[H:, 0],
                             [np.asarray(b_e, f32)[0]]]).astype(f32)
        return np.stack([c0, c1], axis=1)

    We1 = wecat(W_e1, b_e1)
    We2 = wecat(W_e2, b_e2)

    def vcat(W, b_):
        return np.concatenate([np.asarray(W, f32),
                               np.asarray(b_, f32)[None, :]], 0)

    com = dict(
        xT_aug=xT_aug, W_aug=vcat(W_infer, b_infer),
        We1_rep=np.tile(We1[:H, 0], B * NCH)[None, :],
        We2_rep=np.tile(We2[:H, 0], B * NCH)[None, :],
        We1_f=We1, We2_f=We2,
        Wn1a=vcat(np.asarray(W_n1, f32)[:H], b_n1),
        Wn1b=np.asarray(W_n1, f32)[H:],
        Wm1a=vcat(np.asarray(W_m1, f32)[:H], b_m1),
        Wm1b=vcat(np.asarray(W_m1, f32)[H:], np.zeros(NO, f32)),
        Wn2a=vcat(np.asarray(W_n2, f32)[:H], b_n2),
        Wn2b=np.asarray(W_n2, f32)[H:],
        Wm2a=vcat(np.asarray(W_m2, f32)[:H], b_m2),
        Wm2b=vcat(np.asarray(W_m2, f32)[H:], np.zeros(NO, f32)),
    )
    in_maps = []
    for c in range(NC):
        sl = slice(c * GL, (c + 1) * GL)
        xl = np.asarray(x, f32)[:, sl, :].transpose(2, 0, 1).reshape(NI, B * GL)
        m = dict(com)
        m["xT_loc"] = np.concatenate([xl, np.ones((1, B * GL), f32)], 0)
        m["w1T"] = np.ascontiguousarray(w1[sl, :].T)
        m["w2T"] = np.ascontiguousarray(w2[sl, :].T)
        g = np.asarray(bn_gamma, f32)[sl]
        b_ = np.asarray(bn_beta, f32)[sl]
        m["bn_g_nat"] = np.ascontiguousarray(g.reshape(LCH, 128).T)
        m["bn_b_nat"] = np.ascontiguousarray(b_.reshape(LCH, 128).T)
        m["bn_g_row"] = np.ascontiguousarray(g[None, :])
        m["bn_b_row"] = np.ascontiguousarray(b_[None, :])
        in_maps.append(m)
    return in_maps


def kernel(**inputs):
    if "nc" not in _CACHE:
        _CACHE["nc"] = build_program()
    nc = _CACHE["nc"]
    in_maps = _prep_inputs(**inputs)
    res = run_bass_kernel_spmd(nc, in_maps, list(range(NC)))
    parts = [res.results[c]["out"].reshape(B, GL, NO) for c in range(NC)]
    return np.concatenate(parts, axis=1).astype(np.float32)
